# revision 1
# baseline (speedup 1.0000x reference)
"""Trainium2 Bass kernel for nn_LlamaAttention_cam (sparse attention + CaM merge).

Sharding: tensor-parallel over heads across 8 NeuronCores (2 heads/core).
Each core computes its heads' QKV projections, RoPE, masked attention
(start+recent keep mask), CaM rank-1 correction for the last chunk, and a
partial o_proj (its 256 columns of x against the matching 256 rows of Wo^T).
The host sums the 8 partial outputs (the reduction of the head-parallel
o_proj), which replaces the all-reduce.

Matmuls run as float32r (TF32-like, ~1e-4 rel err, 4x faster than fp32 on
the PE array); accumulation is fp32 in PSUM.
"""

import sys

for _p in ("/opt/trn_rl_repo",):
    if _p not in sys.path:
        sys.path.append(_p)

import numpy as np

import concourse.bass as bass
import concourse.mybir as mybir
import concourse.tile as tile
from concourse import bacc, bass_utils

F32 = mybir.dt.float32
F32R = mybir.dt.float32r
AF = mybir.ActivationFunctionType

T = 2048
DM = 2048
H = 16
D = 128
NCORES = 8
HL = H // NCORES          # heads per core = 2
JC = HL * D               # local attn width = 256
SB = 204                  # start keep
RB = 819                  # recent keep
EV = T - RB               # 1229 (first recent key; CaM source row)
LAST = ((T - 1) // 256) * 256   # 1792 — last chunk start
KC = DM // 128            # 16 model-dim chunks
TB = T // 512             # 4 t-blocks of 512
TI = T // 128             # 16 t-chunks of 128
# kept key blocks: (block idx, kept row range within block)
KBLK = [(0, 0, 128), (1, 0, 76), (9, 77, 128)] + [(b, 0, 128) for b in range(10, 16)]
NB = len(KBLK)            # 9


def _build_nc():
    nc = bacc.Bacc("TRN2", target_bir_lowering=False, debug=False,
                   num_devices=NCORES)
    hsT = nc.dram_tensor("hsT", [DM, T], F32R, kind="ExternalInput").ap()
    wqT = nc.dram_tensor("wqT", [DM, JC], F32R, kind="ExternalInput").ap()
    wkT = nc.dram_tensor("wkT", [DM, JC], F32R, kind="ExternalInput").ap()
    wvT = nc.dram_tensor("wvT", [DM, JC], F32R, kind="ExternalInput").ap()
    woT = nc.dram_tensor("woT", [JC, DM], F32R, kind="ExternalInput").ap()
    cosT = nc.dram_tensor("cosT", [D, T], F32, kind="ExternalInput").ap()
    sinTs = nc.dram_tensor("sinTs", [D, T], F32, kind="ExternalInput").ap()
    u2 = nc.dram_tensor("u2", [1, HL], F32, kind="ExternalInput").ap()
    masks = nc.dram_tensor("masks", [128, 2], F32, kind="ExternalInput").ap()
    po = nc.dram_tensor("po", [T, DM], F32, kind="ExternalOutput").ap()
    dbg = nc.dram_tensor("dbg", [1, 16], F32, kind="ExternalOutput").ap()

    with tile.TileContext(nc) as tc:
        with (
            tc.tile_pool(name="big512", bufs=20) as pbig,      # f32r [128,512]: hsT stream + E tiles
            tc.tile_pool(name="wqk", bufs=8) as pwqk,          # f32r [128,128]
            tc.tile_pool(name="wv", bufs=4) as pwv,            # f32r [128,256]
            tc.tile_pool(name="tmp512", bufs=10) as ptmp,      # f32 [128,512] transients
            tc.tile_pool(name="resid", bufs=1) as pres,        # long-lived
            tc.tile_pool(name="rows", bufs=8) as prow,         # small [1,*] tiles
            tc.tile_pool(name="ps", bufs=5, space="PSUM") as pps,
            tc.tile_pool(name="psdn", bufs=2, space="PSUM") as ppsd,
        ):
            # ---- long-lived tiles ----
            cosT_sb = pres.tile([D, T], F32, tag="cos")
            sinTs_sb = pres.tile([D, T], F32, tag="sin")
            nc.sync.dma_start(cosT_sb[:], cosT[:])
            nc.sync.dma_start(sinTs_sb[:], sinTs[:])
            u2_sb = pres.tile([1, HL], F32, tag="u2")
            nc.sync.dma_start(u2_sb[:], u2[:])
            woT_sb = [pres.tile([128, DM], F32R, tag=f"wo{l}", name=f"wo{l}")
                      for l in range(HL)]
            for l in range(HL):
                nc.sync.dma_start(woT_sb[l][:], woT[l * 128:(l + 1) * 128, :])

            ones_f = pres.tile([128, 1], F32, tag="ones_f")
            nc.vector.memset(ones_f[:], 1.0)
            ones = pres.tile([128, 1], F32R, tag="ones")
            nc.vector.tensor_copy(ones[:], ones_f[:])
            # per-partition 0/1 masks for the partial key blocks
            masks_sb = pres.tile([128, 2], F32, tag="masks")
            nc.sync.dma_start(masks_sb[:], masks[:])
            mask1 = masks_sb[:, 0:1]   # rows < 76 kept
            mask9 = masks_sb[:, 1:2]   # rows >= 77 kept

            # rope'd q/k in [d, t] layout; v in [t, d_local] layout
            qrT = [pres.tile([D, T], F32R, tag=f"qrT{l}", name=f"qrT{l}") for l in range(HL)]
            krT = [pres.tile([D, T], F32R, tag=f"krT{l}", name=f"krT{l}") for l in range(HL)]
            vt = [pres.tile([128, JC], F32R, tag=f"vt{i}", name=f"vt{i}") for i in range(TI)]
            outT = [pres.tile([D, T], F32R, tag=f"outT{l}", name=f"outT{l}") for l in range(HL)]

            # ---------------- phase 1+2: projections + rope ----------------
            for tb in range(TB):
                ts5 = slice(tb * 512, tb * 512 + 512)
                hst = [pbig.tile([128, 512], F32R, tag="big", name=f"hst{tb}_{i}") for i in range(KC)]
                for kc in range(KC):
                    nc.sync.dma_start(hst[kc][:], hsT[kc * 128:(kc + 1) * 128, ts5])
                for l in range(HL):
                    for wdram, dstT in ((wqT, qrT[l]), (wkT, krT[l])):
                        psqk = pps.tile([128, 512], F32, tag="ps")
                        for kc in range(KC):
                            wt = pwqk.tile([128, 128], F32R, tag="wqk")
                            nc.sync.dma_start(
                                wt[:], wdram[kc * 128:(kc + 1) * 128,
                                             l * 128:(l + 1) * 128])
                            nc.tensor.matmul(psqk[:], wt[:], hst[kc][:],
                                             start=(kc == 0), stop=(kc == KC - 1))
                        # rope: raw copy, half-swap, combine
                        raw = ptmp.tile([128, 512], F32, tag="tmp")
                        nc.scalar.copy(raw[:], psqk[:])
                        sh = ptmp.tile([128, 512], F32, tag="tmp")
                        nc.sync.dma_start(sh[0:64, :], raw[64:128, :])
                        nc.sync.dma_start(sh[64:128, :], raw[0:64, :])
                        t1 = ptmp.tile([128, 512], F32, tag="tmp")
                        nc.vector.tensor_mul(t1[:], raw[:], cosT_sb[:, ts5])
                        t2 = ptmp.tile([128, 512], F32, tag="tmp")
                        nc.vector.tensor_mul(t2[:], sh[:], sinTs_sb[:, ts5])
                        nc.vector.tensor_add(dstT[:, ts5], t1[:], t2[:])
                # v projection for the 4 t-chunks of this t-block
                for j in range(4):
                    ti = tb * 4 + j
                    psv = pps.tile([128, JC], F32, tag="ps")
                    for kc in range(KC):
                        wvt = pwv.tile([128, JC], F32R, tag="wv")
                        nc.sync.dma_start(wvt[:], wvT[kc * 128:(kc + 1) * 128, :])
                        nc.tensor.matmul(psv[:], hst[kc][:, j * 128:(j + 1) * 128],
                                         wvt[:], start=(kc == 0), stop=(kc == KC - 1))
                    nc.scalar.copy(vt[ti][:], psv[:])

            # ---------------- phase 3: attention per head / t-block ----------------
            for l in range(HL):
                for tb in range(TB):
                    ts5 = slice(tb * 512, tb * 512 + 512)
                    E = []
                    for (b, r0, r1) in KBLK:
                        pst = pps.tile([128, 512], F32, tag="ps")
                        nc.tensor.matmul(pst[:], krT[l][:, b * 128:(b + 1) * 128],
                                         qrT[l][:, ts5], start=True, stop=True)
                        e = pbig.tile([128, 512], F32R, tag="big")
                        nc.scalar.activation(e[:], pst[:], AF.Exp)
                        if r0 != 0 or r1 != 128:
                            m = mask1 if b == 1 else mask9
                            nc.vector.tensor_scalar_mul(e[:], e[:], m)
                        E.append(e)
                    psav = pps.tile([128, 512], F32, tag="ps")
                    psdn = ppsd.tile([1, 512], F32, tag="dn")
                    for bi, (b, r0, r1) in enumerate(KBLK):
                        nc.tensor.matmul(psav[:], vt[b][:, l * D:(l + 1) * D], E[bi][:],
                                         start=(bi == 0), stop=(bi == NB - 1))
                        nc.tensor.matmul(psdn[:], ones[:], E[bi][:],
                                         start=(bi == 0), stop=(bi == NB - 1))
                    dn_sb = prow.tile([1, 512], F32, tag="row512")
                    nc.vector.tensor_copy(dn_sb[:], psdn[:])
                    recip = prow.tile([1, 512], F32, tag="row512")
                    nc.vector.reciprocal(recip[:], dn_sb[:])

                    if tb == TB - 1:
                        # ---- CaM: bernoulli draw + rank-1 merge on t>=1792 ----
                        pssr = ppsd.tile([1, 256], F32, tag="dn")
                        for bi, (b, r0, r1) in enumerate(KBLK[2:]):
                            nc.tensor.matmul(pssr[:], ones[:], E[2 + bi][:, 256:512],
                                             start=(bi == 0), stop=(bi == NB - 3))
                        # E row of key 1229 (block 9, partition 77), t in [1792,2048)
                        erow = prow.tile([1, 256], F32R, tag="row256")
                        nc.sync.dma_start(erow[:], E[2][77:78, 256:512])
                        srec = prow.tile([1, 256], F32, tag="row256")
                        nc.vector.tensor_sub(srec[:], pssr[:], erow[:].bitcast(F32))
                        # scalars at t = 2047 (col 255 of the 256-wide rows)
                        r_last = recip[0:1, 511:512]
                        num = prow.tile([1, 1], F32, tag="sc")
                        nc.vector.tensor_mul(num[:], erow[0:1, 255:256].bitcast(F32), r_last)
                        mean = prow.tile([1, 1], F32, tag="sc")
                        nc.vector.tensor_mul(mean[:], srec[0:1, 255:256], r_last)
                        nc.vector.tensor_scalar_mul(mean[:], mean[:], 1.0 / 818.0)
                        nc.vector.tensor_scalar_add(mean[:], mean[:], 1e-6)
                        um = prow.tile([1, 1], F32, tag="sc")
                        nc.vector.tensor_mul(um[:], u2_sb[0:1, l:l + 1], mean[:])
                        bern = prow.tile([1, 1], F32, tag="sc")
                        nc.vector.tensor_tensor(bern[:], um[:], num[:],
                                                mybir.AluOpType.is_lt)
                        bs = prow.tile([1, 1], F32, tag="sc")
                        nc.vector.tensor_scalar_mul(bs[:], bern[:], 1.0 / RB)
                        coef = prow.tile([1, 256], F32R, tag="row256r")
                        nc.vector.tensor_scalar_mul(coef[:], srec[:], bs[:])
                        dbgrow = prow.tile([1, 8], F32, tag="dbgrow")
                        nc.vector.tensor_copy(dbgrow[0:1, 0:1], num[:])
                        nc.vector.tensor_copy(dbgrow[0:1, 1:2], mean[:])
                        nc.vector.tensor_copy(dbgrow[0:1, 2:3], bern[:])
                        nc.vector.tensor_copy(dbgrow[0:1, 3:4], srec[0:1, 255:256])
                        nc.vector.tensor_copy(dbgrow[0:1, 4:5], um[:])
                        nc.vector.tensor_copy(dbgrow[0:1, 5:6], u2_sb[0:1, l:l + 1])
                        nc.vector.tensor_copy(dbgrow[0:1, 6:7], r_last)
                        nc.vector.tensor_copy(dbgrow[0:1, 7:8], erow[0:1, 255:256].bitcast(F32))
                        nc.sync.dma_start(dbg[0:1, l * 8:(l + 1) * 8], dbgrow[:])
                        vrow = prow.tile([1, D], F32R, tag="vrow")
                        nc.sync.dma_start(vrow[:], vt[EV // 128][77:78,
                                                                 l * D:(l + 1) * D])
                        pscr = pps.tile([128, 256], F32, tag="ps")
                        nc.tensor.matmul(pscr[:], vrow[:], coef[:],
                                         start=True, stop=True)
                    # normalize columns by 1/denom and store as f32r
                    rbf = ptmp.tile([128, 512], F32, tag="tmp")
                    nc.gpsimd.partition_broadcast(rbf[:], recip[:])
                    nc.vector.tensor_mul(outT[l][:, ts5], psav[:], rbf[:])
                    if tb == TB - 1:
                        corr = ptmp.tile([128, 512], F32, tag="tmp")
                        nc.vector.tensor_mul(corr[:, 0:256], pscr[:], rbf[:, 256:512])
                        nc.vector.tensor_add(outT[l][:, 1792:2048],
                                             outT[l][:, 1792:2048], corr[:, 0:256])

            # ---------------- phase 4: partial o_proj ----------------
            for ti in range(TI):
                for mb in range(TB):
                    pso = pps.tile([128, 512], F32, tag="ps")
                    for l in range(HL):
                        nc.tensor.matmul(pso[:], outT[l][:, ti * 128:(ti + 1) * 128],
                                         woT_sb[l][:, mb * 512:(mb + 1) * 512],
                                         start=(l == 0), stop=(l == HL - 1))
                    osb = ptmp.tile([128, 512], F32, tag="tmp")
                    if (ti * TB + mb) % 2 == 0:
                        nc.scalar.copy(osb[:], pso[:])
                    else:
                        nc.vector.tensor_copy(osb[:], pso[:])
                    nc.sync.dma_start(
                        po[ti * 128:(ti + 1) * 128, mb * 512:(mb + 1) * 512], osb[:])

    nc.compile()
    return nc


_NC_CACHE = None


def _get_nc():
    global _NC_CACHE
    if _NC_CACHE is None:
        _NC_CACHE = _build_nc()
    return _NC_CACHE


def make_in_maps(hidden_states, Wq, Wk, Wv, Wo):
    hs = np.asarray(hidden_states, np.float32).reshape(T, DM)
    hs = np.nan_to_num(hs, nan=0.0, posinf=1e4, neginf=-1e4)
    hsT = np.ascontiguousarray(hs.T)
    Wq = np.asarray(Wq, np.float32)
    Wk = np.asarray(Wk, np.float32)
    Wv = np.asarray(Wv, np.float32)
    Wo = np.asarray(Wo, np.float32)

    inv_freq = 1.0 / (10000.0 ** (np.arange(0, D, 2, dtype=np.float32) / D))
    freqs = np.arange(T, dtype=np.float32)[:, None] * inv_freq[None, :]
    emb = np.concatenate([freqs, freqs], axis=-1)          # [T, D]
    cosT = np.ascontiguousarray(np.cos(emb).T.astype(np.float32))
    sinT = np.sin(emb).T.astype(np.float32)
    sinTs = np.ascontiguousarray(
        np.concatenate([-sinT[:D // 2], sinT[D // 2:]], axis=0))

    import jax
    import jax.numpy as jnp
    u_full = np.asarray(
        jax.random.uniform(jax.random.key(42), (1, H), jnp.float32))

    mask_np = np.zeros((128, 2), np.float32)
    mask_np[:76, 0] = 1.0
    mask_np[77:, 1] = 1.0

    scale = 1.0 / np.sqrt(np.float32(D))
    in_maps = []
    for c in range(NCORES):
        js = slice(c * JC, (c + 1) * JC)
        in_maps.append({
            "hsT": hsT,
            "wqT": np.ascontiguousarray(Wq[js, :].T) * scale,
            "wkT": np.ascontiguousarray(Wk[js, :].T),
            "wvT": np.ascontiguousarray(Wv[js, :].T),
            "woT": np.ascontiguousarray(Wo[:, js].T),
            "cosT": cosT,
            "sinTs": sinTs,
            "u2": np.ascontiguousarray(u_full[:, c * HL:(c + 1) * HL]),
            "masks": mask_np,
        })
    return in_maps


def kernel(hidden_states, Wq, Wk, Wv, Wo):
    nc = _get_nc()
    in_maps = make_in_maps(hidden_states, Wq, Wk, Wv, Wo)
    res = bass_utils.run_bass_kernel_spmd(nc, in_maps,
                                          core_ids=list(range(NCORES)))
    out = np.zeros((T, DM), np.float64)
    for c in range(NCORES):
        out += res.results[c]["po"].astype(np.float64)
    out = np.nan_to_num(out.astype(np.float32), nan=0.0, posinf=1e4,
                        neginf=-1e4)
    return out.reshape(1, T, DM)

